# revision 4
# baseline (speedup 1.0000x reference)
"""Trainium2 Bass kernel for the dynamic segment-aggregation module.

Computation per (clip n, channel c):
  pooled[u]  = mean_{t,h,w} x[n,c,u,...]                (U=4 segments)
  z          = relu(BN(pooled @ W1^T))                  (tiny MLP, eval-mode BN)
  kern       = softmax(z @ W2^T)                        (K=3 taps)
  out[u]     = kern[0]*x[u-1] + kern[1]*x[u] + kern[2]*x[u+1]   (zero-padded)

Sharding: data-parallel over the 8 clips -> 1 clip (4 segments) per NeuronCore.
Per-core layout: channels on the 128 SBUF partitions (2 groups of 128), free
dim = (u, t-quarter * h * w).  Pooling rides the ScalarE activation
accumulator, the 3-tap blend is ScalarE scale-pass + VectorE
scalar_tensor_tensor MACs.  The BN affine (s = gamma*rsqrt(var+eps),
t = beta - mean*s) and the 1/THW pooling mean are folded host-side into a
single packed small-weights tensor so the device sees one tiny DMA.
"""

import numpy as np

import concourse.bass as bass
import concourse.bacc as bacc
import concourse.tile as tile
from concourse import mybir
from concourse.bass_utils import run_bass_kernel_spmd

U = 4          # segments per clip
C = 256        # channels
T, H, W = 8, 28, 28
THW = T * H * W            # 6272
NQ = 4                     # t-quarters per channel-group
FQ = THW // NQ             # 1568
D = 8                      # MLP hidden dim (U * alpha)
K = 3                      # conv taps
EPS = 1e-5
N_CORES = 8
NCG = C // 128             # channel groups per core

# packed small-weights layout: [W1*(1/THW) (D*U) | W2 (K*D) | s (D) | t (D)]
NPACK = D * U + K * D + D + D    # 72

FP32 = mybir.dt.float32

_nc_cache = None
last_results = None        # BassKernelResults of the most recent run (for test.py)


def _bcast_ap(ap, parts=128):
    """DRAM AP replicated across `parts` partitions (partition stride 0)."""
    return bass.AP(tensor=ap.tensor, offset=ap.offset, ap=[[0, parts]] + list(ap.ap))


def _build_nc():
    nc = bacc.Bacc(None, target_bir_lowering=False)
    x_h = nc.declare_dram_parameter("x", [U, C, THW], FP32, isOutput=False)
    wp_h = nc.declare_dram_parameter("wpack", [NPACK], FP32, isOutput=False)
    out_h = nc.declare_dram_parameter("out", [U, C, THW], FP32, isOutput=True)

    xg = x_h[:].rearrange("u c f -> c u f")      # [C, U, THW]
    og = out_h[:].rearrange("u c f -> c u f")

    AX = mybir.AxisListType
    OP = mybir.AluOpType
    AF = mybir.ActivationFunctionType

    with tile.TileContext(nc) as tc:
        with (
            tc.tile_pool(name="xp", bufs=5) as xp,
            tc.tile_pool(name="outp", bufs=2) as outp,
            tc.tile_pool(name="t1p", bufs=2) as t1p,
            tc.tile_pool(name="small", bufs=1) as small,
            tc.tile_pool(name="mlp", bufs=2) as mlp,
        ):
            # one tiny DMA for every per-core-replicated constant
            wpk = small.tile([128, NPACK], FP32)
            nc.gpsimd.dma_start(out=wpk, in_=_bcast_ap(wp_h[:]))
            w1sb = wpk[:, 0:D * U].rearrange("p (d u) -> p d u", d=D)       # [128,D,U]
            w2sb = wpk[:, D * U:D * U + K * D].rearrange(
                "p (k d) -> p k d", k=K)                                    # [128,K,D]
            s_t = wpk[:, D * U + K * D:D * U + K * D + D]                   # [128,D]
            o_t = wpk[:, D * U + K * D + D:NPACK]                           # [128,D]

            for g in range(NCG):
                c0 = g * 128
                # ---- load quarters; pooling rides ScalarE accumulate ----
                P = mlp.tile([128, U, NQ], FP32, tag="P")
                slabs = []
                for q in range(NQ):
                    sl = xp.tile([128, U, FQ], FP32, tag="xslab")
                    slabs.append(sl)
                    nc.sync.dma_start(
                        out=sl, in_=xg[c0:c0 + 128, :, q * FQ:(q + 1) * FQ]
                    )
                    for u in range(U):
                        nc.scalar.activation(
                            out=sl[:, u, :], in_=sl[:, u, :], func=AF.Copy,
                            accum_out=P[:, u, q:q + 1],
                        )

                # ---- tiny generator MLP (channels stay on partitions) ----
                pooled = mlp.tile([128, U], FP32, tag="pooled")
                nc.vector.reduce_sum(out=pooled, in_=P, axis=AX.X)
                z = mlp.tile([128, D], FP32, tag="z")
                nc.vector.tensor_scalar_mul(
                    out=z, in0=w1sb[:, :, 0], scalar1=pooled[:, 0:1]
                )
                for u in range(1, U):
                    nc.vector.scalar_tensor_tensor(
                        out=z, in0=w1sb[:, :, u], scalar=pooled[:, u:u + 1],
                        in1=z, op0=OP.mult, op1=OP.add,
                    )
                # z = relu(z * s + t)
                nc.vector.tensor_mul(out=z, in0=z, in1=s_t)
                nc.vector.tensor_add(out=z, in0=z, in1=o_t)
                nc.vector.tensor_scalar_max(out=z, in0=z, scalar1=0.0)

                logit = mlp.tile([128, K], FP32, tag="logit")
                nc.vector.tensor_scalar_mul(
                    out=logit, in0=w2sb[:, :, 0], scalar1=z[:, 0:1]
                )
                for d in range(1, D):
                    nc.vector.scalar_tensor_tensor(
                        out=logit, in0=w2sb[:, :, d], scalar=z[:, d:d + 1],
                        in1=logit, op0=OP.mult, op1=OP.add,
                    )
                mx = mlp.tile([128, 1], FP32, tag="mx")
                nc.vector.reduce_max(out=mx, in_=logit, axis=AX.X)
                nc.vector.tensor_scalar_mul(out=mx, in0=mx, scalar1=-1.0)
                nc.scalar.activation(
                    out=logit, in_=logit, func=AF.Exp, bias=mx[:, 0:1]
                )
                ssum = mlp.tile([128, 1], FP32, tag="ssum")
                nc.vector.reduce_sum(out=ssum, in_=logit, axis=AX.X)
                nc.vector.reciprocal(out=ssum, in_=ssum)
                kern = mlp.tile([128, K], FP32, tag="kern")
                nc.vector.tensor_scalar_mul(out=kern, in0=logit, scalar1=ssum[:, 0:1])
                k0, k1, k2 = kern[:, 0:1], kern[:, 1:2], kern[:, 2:3]

                # ---- 3-tap blend along u:  ScalarE edge-tap, VectorE MACs ----
                for q in range(NQ):
                    sl = slabs[q]
                    ot = outp.tile([128, U, FQ], FP32, tag="outslab")
                    # u = 0: k1*x0 + k2*x1
                    t1 = t1p.tile([128, FQ], FP32, tag="t1")
                    nc.scalar.activation(out=t1, in_=sl[:, 1, :], func=AF.Copy, scale=k2)
                    nc.vector.scalar_tensor_tensor(
                        out=ot[:, 0, :], in0=sl[:, 0, :], scalar=k1, in1=t1,
                        op0=OP.mult, op1=OP.add,
                    )
                    # u = 1, 2: k0*x[u-1] + k1*x[u] + k2*x[u+1]
                    for u in (1, 2):
                        t1 = t1p.tile([128, FQ], FP32, tag="t1")
                        nc.scalar.activation(
                            out=t1, in_=sl[:, u + 1, :], func=AF.Copy, scale=k2
                        )
                        nc.vector.scalar_tensor_tensor(
                            out=t1, in0=sl[:, u, :], scalar=k1, in1=t1,
                            op0=OP.mult, op1=OP.add,
                        )
                        nc.vector.scalar_tensor_tensor(
                            out=ot[:, u, :], in0=sl[:, u - 1, :], scalar=k0, in1=t1,
                            op0=OP.mult, op1=OP.add,
                        )
                    # u = 3: k0*x2 + k1*x3
                    t1 = t1p.tile([128, FQ], FP32, tag="t1")
                    nc.scalar.activation(out=t1, in_=sl[:, 3, :], func=AF.Copy, scale=k1)
                    nc.vector.scalar_tensor_tensor(
                        out=ot[:, 3, :], in0=sl[:, 2, :], scalar=k0, in1=t1,
                        op0=OP.mult, op1=OP.add,
                    )
                    nc.sync.dma_start(
                        out=og[c0:c0 + 128, :, q * FQ:(q + 1) * FQ], in_=ot
                    )
    nc.finalize()
    return nc


def _get_nc():
    global _nc_cache
    if _nc_cache is None:
        _nc_cache = _build_nc()
    return _nc_cache


def _pack_small(W1, bn_gamma, bn_beta, bn_mean, bn_var, W2):
    W1 = np.asarray(W1, np.float32)
    W2 = np.asarray(W2, np.float32)
    gam = np.asarray(bn_gamma, np.float32)
    bet = np.asarray(bn_beta, np.float32)
    mea = np.asarray(bn_mean, np.float32)
    var = np.asarray(bn_var, np.float32)
    s = (gam / np.sqrt(var + np.float32(EPS))).astype(np.float32)
    t = (bet - mea * s).astype(np.float32)
    w1s = (W1 * np.float32(1.0 / THW)).astype(np.float32)
    return np.concatenate(
        [w1s.reshape(-1), W2.reshape(-1), s, t]
    ).astype(np.float32)


def kernel(x, W1, bn_gamma, bn_beta, bn_mean, bn_var, W2):
    global last_results
    nc = _get_nc()
    x = np.ascontiguousarray(np.asarray(x, dtype=np.float32)).reshape(
        N_CORES, U, C, THW
    )
    wpack = _pack_small(W1, bn_gamma, bn_beta, bn_mean, bn_var, W2)
    in_maps = [{"x": x[i], "wpack": wpack} for i in range(N_CORES)]
    last_results = run_bass_kernel_spmd(nc, in_maps, list(range(N_CORES)))
    out = np.stack([last_results.results[i]["out"] for i in range(N_CORES)])
    return out.reshape(N_CORES * U, C, T, H, W)


# revision 7
# speedup vs baseline: 1.0635x; 1.0635x over previous
"""Trainium2 Bass kernel for the dynamic segment-aggregation module.

Computation per (clip n, channel c):
  pooled[u]  = mean_{t,h,w} x[n,c,u,...]                (U=4 segments)
  z          = relu(BN(pooled @ W1^T))                  (tiny MLP, eval-mode BN)
  kern       = softmax(z @ W2^T)                        (K=3 taps)
  out[u]     = kern[0]*x[u-1] + kern[1]*x[u] + kern[2]*x[u+1]   (zero-padded)

Sharding: data-parallel over the 8 clips -> 1 clip (4 segments) per NeuronCore.
Per-core layout: channels on the 128 SBUF partitions (2 groups of 128), free
dim = (u, t-quarter * h * w).  Pooling rides the ScalarE activation
accumulator, the 3-tap blend is ScalarE scale-pass + VectorE
scalar_tensor_tensor MACs.  The BN affine (s = gamma*rsqrt(var+eps),
t = beta - mean*s) and the 1/THW pooling mean are folded host-side into a
single packed small-weights tensor so the device sees one tiny DMA.
"""

import numpy as np

import concourse.bass as bass
import concourse.bacc as bacc
import concourse.tile as tile
from concourse import mybir
from concourse.bass_utils import run_bass_kernel_spmd

U = 4          # segments per clip
C = 256        # channels
T, H, W = 8, 28, 28
THW = T * H * W            # 6272
NQ = 4                     # t-quarters per channel-group
FQ = THW // NQ             # 1568
D = 8                      # MLP hidden dim (U * alpha)
K = 3                      # conv taps
EPS = 1e-5
N_CORES = 8
NCG = C // 128             # channel groups per core

# packed small-weights layout: [W1*(1/THW) (D*U) | W2 (K*D) | s (D) | t (D)]
NPACK = D * U + K * D + D + D    # 72

FP32 = mybir.dt.float32

_nc_cache = None
last_results = None        # BassKernelResults of the most recent run (for test.py)


def _bcast_ap(ap, parts=128):
    """DRAM AP replicated across `parts` partitions (partition stride 0)."""
    return bass.AP(tensor=ap.tensor, offset=ap.offset, ap=[[0, parts]] + list(ap.ap))


def _build_nc():
    nc = bacc.Bacc(None, target_bir_lowering=False)
    x_h = nc.declare_dram_parameter("x", [U, C, THW], FP32, isOutput=False)
    wp_h = nc.declare_dram_parameter("wpack", [NPACK], FP32, isOutput=False)
    out_h = nc.declare_dram_parameter("out", [U, C, THW], FP32, isOutput=True)

    xg = x_h[:].rearrange("u c f -> c u f")      # [C, U, THW]
    og = out_h[:].rearrange("u c f -> c u f")

    AX = mybir.AxisListType
    OP = mybir.AluOpType
    AF = mybir.ActivationFunctionType

    with tile.TileContext(nc) as tc:
        with (
            tc.tile_pool(name="xp", bufs=5) as xp,
            tc.tile_pool(name="outp", bufs=2) as outp,
            tc.tile_pool(name="t1p", bufs=2) as t1p,
            tc.tile_pool(name="small", bufs=1) as small,
            tc.tile_pool(name="mlp", bufs=2) as mlp,
        ):
            # one tiny DMA for every per-core-replicated constant
            wpk = small.tile([128, NPACK], FP32)
            nc.gpsimd.dma_start(out=wpk, in_=_bcast_ap(wp_h[:]))
            w1sb = wpk[:, 0:D * U].rearrange("p (d u) -> p d u", d=D)       # [128,D,U]
            w2sb = wpk[:, D * U:D * U + K * D].rearrange(
                "p (k d) -> p k d", k=K)                                    # [128,K,D]
            s_t = wpk[:, D * U + K * D:D * U + K * D + D]                   # [128,D]
            o_t = wpk[:, D * U + K * D + D:NPACK]                           # [128,D]

            for g in range(NCG):
                c0 = g * 128
                # ---- load quarters; pooling rides ScalarE accumulate ----
                P = mlp.tile([128, U, NQ], FP32, tag="P")
                slabs = []
                for q in range(NQ):
                    sl = xp.tile([128, U, FQ], FP32, tag="xslab")
                    slabs.append(sl)
                    nc.sync.dma_start(
                        out=sl, in_=xg[c0:c0 + 128, :, q * FQ:(q + 1) * FQ]
                    )
                    # pooling: split between ScalarE (activation accumulator)
                    # and VectorE (tensor_scalar accumulator) for balance
                    for u in (0, 1):
                        nc.scalar.activation(
                            out=sl[:, u, :], in_=sl[:, u, :], func=AF.Copy,
                            accum_out=P[:, u, q:q + 1],
                        )
                    for u in (2, 3):
                        nc.vector.tensor_scalar(
                            out=sl[:, u, :], in0=sl[:, u, :], scalar1=1.0,
                            scalar2=0.0, op0=OP.mult, op1=OP.add,
                            accum_out=P[:, u, q:q + 1],
                        )

                # ---- tiny generator MLP (channels stay on partitions) ----
                pooled = mlp.tile([128, U], FP32, tag="pooled")
                nc.vector.reduce_sum(out=pooled, in_=P, axis=AX.X)
                z = mlp.tile([128, D], FP32, tag="z")
                nc.vector.tensor_scalar_mul(
                    out=z, in0=w1sb[:, :, 0], scalar1=pooled[:, 0:1]
                )
                for u in range(1, U):
                    nc.vector.scalar_tensor_tensor(
                        out=z, in0=w1sb[:, :, u], scalar=pooled[:, u:u + 1],
                        in1=z, op0=OP.mult, op1=OP.add,
                    )
                # z = relu(z * s + t)
                nc.vector.tensor_mul(out=z, in0=z, in1=s_t)
                nc.vector.tensor_add(out=z, in0=z, in1=o_t)
                nc.vector.tensor_scalar_max(out=z, in0=z, scalar1=0.0)

                logit = mlp.tile([128, K], FP32, tag="logit")
                nc.vector.tensor_scalar_mul(
                    out=logit, in0=w2sb[:, :, 0], scalar1=z[:, 0:1]
                )
                for d in range(1, D):
                    nc.vector.scalar_tensor_tensor(
                        out=logit, in0=w2sb[:, :, d], scalar=z[:, d:d + 1],
                        in1=logit, op0=OP.mult, op1=OP.add,
                    )
                mx = mlp.tile([128, 1], FP32, tag="mx")
                nc.vector.reduce_max(out=mx, in_=logit, axis=AX.X)
                nc.vector.tensor_scalar_mul(out=mx, in0=mx, scalar1=-1.0)
                nc.scalar.activation(
                    out=logit, in_=logit, func=AF.Exp, bias=mx[:, 0:1]
                )
                ssum = mlp.tile([128, 1], FP32, tag="ssum")
                nc.vector.reduce_sum(out=ssum, in_=logit, axis=AX.X)
                nc.vector.reciprocal(out=ssum, in_=ssum)
                kern = mlp.tile([128, K], FP32, tag="kern")
                nc.vector.tensor_scalar_mul(out=kern, in0=logit, scalar1=ssum[:, 0:1])
                k0, k1, k2 = kern[:, 0:1], kern[:, 1:2], kern[:, 2:3]

                # ---- 3-tap blend along u:  ScalarE edge-tap, VectorE MACs ----
                for q in range(NQ):
                    sl = slabs[q]
                    ot = outp.tile([128, U, FQ], FP32, tag="outslab")
                    # u = 0: k1*x0 + k2*x1
                    t1 = t1p.tile([128, FQ], FP32, tag="t1")
                    nc.scalar.activation(out=t1, in_=sl[:, 1, :], func=AF.Copy, scale=k2)
                    nc.vector.scalar_tensor_tensor(
                        out=ot[:, 0, :], in0=sl[:, 0, :], scalar=k1, in1=t1,
                        op0=OP.mult, op1=OP.add,
                    )
                    # u = 1, 2: k0*x[u-1] + k1*x[u] + k2*x[u+1]
                    for u in (1, 2):
                        t1 = t1p.tile([128, FQ], FP32, tag="t1")
                        nc.scalar.activation(
                            out=t1, in_=sl[:, u + 1, :], func=AF.Copy, scale=k2
                        )
                        nc.vector.scalar_tensor_tensor(
                            out=t1, in0=sl[:, u, :], scalar=k1, in1=t1,
                            op0=OP.mult, op1=OP.add,
                        )
                        nc.vector.scalar_tensor_tensor(
                            out=ot[:, u, :], in0=sl[:, u - 1, :], scalar=k0, in1=t1,
                            op0=OP.mult, op1=OP.add,
                        )
                    # u = 3: k0*x2 + k1*x3
                    t1 = t1p.tile([128, FQ], FP32, tag="t1")
                    nc.scalar.activation(out=t1, in_=sl[:, 3, :], func=AF.Copy, scale=k1)
                    nc.vector.scalar_tensor_tensor(
                        out=ot[:, 3, :], in0=sl[:, 2, :], scalar=k0, in1=t1,
                        op0=OP.mult, op1=OP.add,
                    )
                    # stores ride the (otherwise idle) GpSimd SWDGE queue so the
                    # Sync HWDGE queue stays free for the next group's loads
                    nc.gpsimd.dma_start(
                        out=og[c0:c0 + 128, :, q * FQ:(q + 1) * FQ], in_=ot
                    )
    nc.finalize()
    return nc


def _get_nc():
    global _nc_cache
    if _nc_cache is None:
        _nc_cache = _build_nc()
    return _nc_cache


def _pack_small(W1, bn_gamma, bn_beta, bn_mean, bn_var, W2):
    W1 = np.asarray(W1, np.float32)
    W2 = np.asarray(W2, np.float32)
    gam = np.asarray(bn_gamma, np.float32)
    bet = np.asarray(bn_beta, np.float32)
    mea = np.asarray(bn_mean, np.float32)
    var = np.asarray(bn_var, np.float32)
    s = (gam / np.sqrt(var + np.float32(EPS))).astype(np.float32)
    t = (bet - mea * s).astype(np.float32)
    w1s = (W1 * np.float32(1.0 / THW)).astype(np.float32)
    return np.concatenate(
        [w1s.reshape(-1), W2.reshape(-1), s, t]
    ).astype(np.float32)


def kernel(x, W1, bn_gamma, bn_beta, bn_mean, bn_var, W2):
    global last_results
    nc = _get_nc()
    x = np.ascontiguousarray(np.asarray(x, dtype=np.float32)).reshape(
        N_CORES, U, C, THW
    )
    wpack = _pack_small(W1, bn_gamma, bn_beta, bn_mean, bn_var, W2)
    in_maps = [{"x": x[i], "wpack": wpack} for i in range(N_CORES)]
    last_results = run_bass_kernel_spmd(nc, in_maps, list(range(N_CORES)))
    out = np.stack([last_results.results[i]["out"] for i in range(N_CORES)])
    return out.reshape(N_CORES * U, C, T, H, W)
